# revision 1
# baseline (speedup 1.0000x reference)
"""BitNet ternary-gate dense layer on 8 Trainium2 NeuronCores.

Computes: out = (noise @ ternary(weight).T) * sigma
  where ternary(w) = sign(w) * (|w| > 0.7*mean(|w|)).

Strategy (tensor-parallel over weight rows / output dim):
  - Shard weight rows O=8192 -> 1024 per core; replicate noise.
  - Host: transpose noise -> nt [R, B] and each weight shard -> wt [R, O_sh];
    compute the scalar threshold on host (float64 mean, negligible error).
  - Device (per core): quantize wt to {-1,0,+1} on VectorE (two compares +
    subtract), run the matmul on TensorE in float32r (full bf16-rate on TRN2,
    ~11-bit mantissa rounding of the noise operand only - the ternary weights
    are exact in any float format), accumulate fp32 in PSUM over R=4096,
    scale by sigma on ScalarE, DMA out the transposed output shard [O_sh, B].
  - Host: transpose/concat the 8 output shards -> [B, O].

The matmul maps out.T tiles: psum[o:128, b:512] += wq[k][:,o*128:+128].T @ nt
(k partition dim 128; weight tile stationary, noise chunk moving, N=512).
8 PSUM banks hold 8 o-tile groups concurrently so the PE can start while the
quantizer is still producing later k-chunks.
"""

import sys

_TRN = "/opt/trn_rl_repo"
if _TRN not in sys.path:
    sys.path.insert(0, _TRN)

import numpy as np

import concourse.bass as bass  # noqa: F401
import concourse.tile as tile
from concourse import bacc, mybir
from concourse.bass_utils import run_bass_kernel_spmd

B, R, O = 2048, 4096, 8192
NCORES = 8
O_SH = O // NCORES  # 1024
P = 128
KT = R // P  # 32 k-tiles
O_TILES = O_SH // P  # 8
NB = 512  # moving free dim (one PSUM bank of fp32)
B_BLKS = B // NB  # 4


def build(loop_n=None):
    f32 = mybir.dt.float32
    f32r = mybir.dt.float32r
    bf16 = mybir.dt.bfloat16
    op = mybir.AluOpType

    nc = bacc.Bacc("TRN2", target_bir_lowering=False, debug=False, num_devices=NCORES)
    nt = nc.dram_tensor("nt", [R, B], f32r, kind="ExternalInput")
    wt = nc.dram_tensor("wt", [R, O_SH], f32, kind="ExternalInput")
    sc = nc.dram_tensor("sc", [1, 2], f32, kind="ExternalInput")  # [thresh, sigma]
    outT = nc.dram_tensor("outT", [O_SH, B], f32, kind="ExternalOutput")

    nt_v = nt.ap().rearrange("(ko p) b -> p ko b", p=P)
    wt_v = wt.ap().rearrange("(ko p) o -> p ko o", p=P)
    outT_v = outT.ap()

    with tile.TileContext(nc) as tc:
        with (
            tc.tile_pool(name="consts", bufs=1) as consts,
            tc.tile_pool(name="wqp", bufs=1) as wqp,
            tc.tile_pool(name="rawp", bufs=3) as rawp,
            tc.tile_pool(name="qtmp", bufs=3) as qtmp,
            tc.tile_pool(name="ntp", bufs=6) as ntp,
            tc.tile_pool(name="obp", bufs=6) as obp,
            tc.tile_pool(name="psp", bufs=1, space="PSUM") as psp,
        ):
            sct = consts.tile([P, 2], f32)
            nc.sync.dma_start(sct[:], sc.ap().partition_broadcast(P))
            t_ap = sct[:, 0:1]
            sig_ap = sct[:, 1:2]
            negt = consts.tile([P, 1], f32)
            nc.vector.tensor_scalar_mul(negt[:], t_ap, -1.0)

            def body():
                # --- quantize weight shard: wq[k] in {-1,0,+1} as float32r ---
                wq = []
                for k in range(KT):
                    rawt = rawp.tile([P, O_SH], f32, tag="raw")
                    nc.sync.dma_start(rawt[:], wt_v[:, k])
                    qa = qtmp.tile([P, O_SH], bf16, tag="qa")
                    qb = qtmp.tile([P, O_SH], bf16, tag="qb")
                    nc.vector.tensor_scalar(qa[:], rawt[:], t_ap, None, op.is_gt)
                    nc.vector.tensor_scalar(qb[:], rawt[:], negt[:], None, op.is_lt)
                    wq_k = wqp.tile([P, O_SH], f32r, tag=f"wq{k}", name=f"wq{k}")
                    nc.vector.tensor_tensor(wq_k[:], qa[:], qb[:], op.subtract)
                    wq.append(wq_k)

                # --- matmul: psum[o:128, b:512] += wq_k[:, o].T @ nt_chunk ---
                for bb in range(B_BLKS):
                    psums = [
                        psp.tile([P, NB], f32, tag=f"ps{o}", name=f"ps{o}")
                        for o in range(O_TILES)
                    ]
                    for k in range(KT):
                        ntc = ntp.tile([P, NB], f32r, tag="ntc")
                        nc.sync.dma_start(ntc[:], nt_v[:, k, bb * NB : (bb + 1) * NB])
                        for o in range(O_TILES):
                            nc.tensor.matmul(
                                psums[o],
                                wq[k][:, o * P : (o + 1) * P],
                                ntc[:],
                                start=(k == 0),
                                stop=(k == KT - 1),
                            )
                    for o in range(O_TILES):
                        ot = obp.tile([P, NB], f32, tag="ot")
                        nc.scalar.mul(ot[:], psums[o][:], sig_ap)
                        nc.sync.dma_start(
                            outT_v[o * P : (o + 1) * P, bb * NB : (bb + 1) * NB], ot[:]
                        )

            if loop_n:
                with tc.For_i(0, loop_n, 1):
                    body()
            else:
                body()
    nc.finalize()
    return nc


_NC_CACHE = {}


def _get_nc():
    if "nc" not in _NC_CACHE:
        _NC_CACHE["nc"] = build()
    return _NC_CACHE["nc"]


def _threshold(weight: np.ndarray) -> np.float32:
    """0.7 * mean(|w|), matching the fp32 jax-CPU reference as closely as
    possible: try jax on CPU (bitwise-identical reduction), else float64."""
    try:
        import jax
        import jax.numpy as jnp

        cpu = jax.devices("cpu")[0]
        with jax.default_device(cpu):
            t = 0.7 * jnp.mean(jnp.abs(jnp.asarray(weight)))
        return np.float32(t)
    except Exception:
        return np.float32(0.7 * np.mean(np.abs(weight).astype(np.float64)))


def kernel(noise: np.ndarray, weight: np.ndarray, sigma: np.ndarray) -> np.ndarray:
    noise = np.asarray(noise, dtype=np.float32)
    weight = np.asarray(weight, dtype=np.float32)
    thresh = _threshold(weight)
    sc = np.array([[thresh, np.float32(sigma)]], dtype=np.float32)

    nt = np.ascontiguousarray(noise.T)  # [R, B]
    in_maps = []
    for c in range(NCORES):
        wt_c = np.ascontiguousarray(weight[c * O_SH : (c + 1) * O_SH, :].T)  # [R, O_sh]
        in_maps.append({"nt": nt, "wt": wt_c, "sc": sc})

    nc = _get_nc()
    res = run_bass_kernel_spmd(nc, in_maps, core_ids=list(range(NCORES)), trace=False)

    out = np.empty((B, O), dtype=np.float32)
    for c in range(NCORES):
        out[:, c * O_SH : (c + 1) * O_SH] = res.results[c]["outT"].T
    return out
